# revision 7
# baseline (speedup 1.0000x reference)
"""Causal single-head attention on 8 Trainium2 NeuronCores (batch-parallel), v2.

Problem (nn_Head): x[32,1024,256] f32, Wk/Wq/Wv[64,256] f32.
  q/k/v = x @ W.T ; wei = softmax(causal(q @ k.T / 8)) ; out = wei @ v.

Sharding: B=32 split 4-per-core across 8 cores; weights replicated.

v2 changes vs baseline:
  - weights DMA first on the scalar queue (no longer queued behind x).
  - all items' x loads issued upfront (co0 on sync, co1 on gpsimd queues).
  - PE warmup dummy matmuls so real matmuls run at 2.4 GHz (HAM warm).
  - scores for si=4,5 share one PSUM tile (one exp), si=6,7 share one bank
    (one exp): 6 ACTIVATEs/item instead of 8 (ScalarE is the pacing engine).
  - diagonal mask-multiplies moved to the idle Pool (gpsimd) engine.
  - explicit software pipeline: next item's projections are emitted between
    this item's score matmuls; next item's first score matmul is emitted
    before this item's PV tail so ScalarE never starves.
  - outputs stored per-half, all on the sync queue (inputs done by then).
"""

import numpy as np
import ml_dtypes

B, T, C, HS = 32, 1024, 256, 64
NCORES = 8
BPC = B // NCORES  # batch items per core
P = 128            # partitions / row-tile
NT = T // P        # 8 row tiles per item
CO = C // P        # 2 contraction chunks for projections
TCH = 512          # matmul free-dim chunk (one PSUM bank of f32)
N_WARMUP = 5       # dummy matmuls (N=512) to warm the PE clock

_cached = {}


def _build():
    import concourse.tile as tile
    from concourse import bacc, mybir

    bf16 = mybir.dt.bfloat16
    f32 = mybir.dt.float32
    Exp = mybir.ActivationFunctionType.Exp
    Mult = mybir.AluOpType.mult

    nc = bacc.Bacc(
        "TRN2",
        target_bir_lowering=False,
        debug=False,
        num_devices=NCORES,
    )

    xT = nc.dram_tensor("xT", [BPC, C, T], bf16, kind="ExternalInput")
    # packed weights, one DMA: per partition p the 768 bf16 columns are
    # [wA(co0)|wA(co1)|wB(co0)|wB(co1)|wV(co0)|wV(co1)|mask]
    wcat = nc.dram_tensor("wcat", [P, 768], bf16, kind="ExternalInput")
    out = nc.dram_tensor("out", [BPC, T, HS], f32, kind="ExternalOutput")

    # score-group layout: group id -> (si list, psum cols, act window)
    #   groups 0..3 hold a single si in a [P, 1024] tile (2 banks), valid
    #   t from 128*si, exp reads exactly the causal span.
    #   group 4 = {si4, si5} in [P, 1024]: si4 at cols 0:512 (t=512..1024),
    #   si5 at cols 512:1024 (same t window; valid from t=640) -> one exp
    #   of 1024 cols (128 garbage, never read downstream).
    #   group 5 = {si6, si7} in [P, 512] (1 bank): si6 at 0:256
    #   (t=768..1024), si7 at 256:384 (t=896..1024) -> one exp of 384 cols.

    with tile.TileContext(nc) as tc:
        with (
            tc.tile_pool(name="consts", bufs=1) as consts,
            tc.tile_pool(name="xin", bufs=4) as xin,
            tc.tile_pool(name="ab", bufs=4) as abp,
            tc.tile_pool(name="vau", bufs=3) as vaup,
            tc.tile_pool(name="expw", bufs=2) as expwp,
            tc.tile_pool(name="outp", bufs=3) as outp,
            tc.tile_pool(name="ps_big", bufs=3, space="PSUM") as ps_big,
            tc.tile_pool(name="ps_sm", bufs=2, space="PSUM") as ps_sm,
        ):
            # ---- input DMAs ---------------------------------------------
            # packed weights blob: one trigger, first thing on the sync
            # queue so it lands before the (larger) x pieces behind it.
            wcat_sb = consts.tile([P, 768], bf16, tag="wcat")
            nc.sync.dma_start(wcat_sb, wcat[:, :])

            def wA_co(co):
                return wcat_sb[:, co * P:(co + 1) * P]

            def wB_co(co):
                return wcat_sb[:, 256 + co * P:256 + (co + 1) * P]

            def wV_co(co):
                return wcat_sb[:, 512 + co * HS:512 + (co + 1) * HS]

            mask_sb = consts.tile([P, P], bf16, tag="mask")
            nc.gpsimd.tensor_copy(mask_sb, wcat_sb[:, 640:768])

            # warmup source first so the gpsimd memset isn't queued behind
            # the DMA trigger instructions
            dummy_src = consts.tile([P, TCH], bf16, tag="dummy")
            nc.gpsimd.memset(dummy_src, 0.0)

            # x loads: co0 on the sync HW queue, co1 on the scalar HW
            # queue.  The DMA engines round-robin across every queued
            # transfer, so item 0 (t-halved, 4 pieces) goes up alone;
            # each later item's DMA is WAW-gated on a tiny copy that
            # depends on the PREVIOUS item's A-cast, which serializes
            # the items on the wire in the order compute needs them.
            xT_tiles = []
            gate_src = {}
            for it in range(BPC):
                t = xin.tile([P, CO, T], bf16, tag="xT", name=f"xT{it}")
                r = xT[it].rearrange("(co p) t -> p co t", p=P)
                xT_tiles.append(t)
            for h in range(2):
                nc.sync.dma_start(
                    xT_tiles[0][:, 0:1, h * TCH:(h + 1) * TCH],
                    xT[0].rearrange("(co p) t -> p co t", p=P)[
                        :, 0:1, h * TCH:(h + 1) * TCH],
                )
                nc.scalar.dma_start(
                    xT_tiles[0][:, 1:2, h * TCH:(h + 1) * TCH],
                    xT[0].rearrange("(co p) t -> p co t", p=P)[
                        :, 1:2, h * TCH:(h + 1) * TCH],
                )

            def load_gated(it, gate_ap, co1_eng=None):
                # items 2/3 go sync-only: a late-gated trigger in the
                # scalar engine's stream would block the exp pipeline
                # (in-order engine).  item 1's gate fires before the
                # first exp, so its co1 half may use the scalar queue.
                t = xT_tiles[it]
                r = xT[it].rearrange("(co p) t -> p co t", p=P)
                nc.gpsimd.tensor_copy(t[:, 0, 0:2], gate_ap)
                nc.gpsimd.tensor_copy(t[:, 1, 0:2], gate_ap)
                nc.sync.dma_start(t[:, 0:1, :], r[:, 0:1, :])
                (co1_eng or nc.sync).dma_start(t[:, 1:2, :], r[:, 1:2, :])

            # item 1: gated on item 0's own co1 data having landed.
            # co1 stays on sync too: a gated trigger at the top of the
            # scalar stream would delay the ACT table load behind it.
            load_gated(1, xT_tiles[0][:, 1, T - 2:T])

            # ---- PE warmup ----------------------------------------------
            ps_warm = ps_big.tile([P, 2, TCH], f32, tag="ps", name="warm")
            for w in range(N_WARMUP):
                nc.tensor.matmul(
                    ps_warm[:, w % 2, :],
                    dummy_src[:, 0:P],
                    dummy_src,
                    start=True,
                    stop=True,
                )

            # ---- per-item emitters --------------------------------------
            A_sb = {}
            B_sb = {}
            vaug = {}
            expw = {}   # (item, grp) -> ew tile
            po = {}     # (item, half) -> psum tile
            osb = {}

            def emit_projA(i, ps=None):
                if ps is None:
                    ps = ps_big.tile([P, 2 * TCH], f32, tag="ps",
                                     name=f"pA{i}")
                for h in range(2):
                    for co in range(CO):
                        nc.tensor.matmul(
                            ps[:, h * TCH:(h + 1) * TCH],
                            wA_co(co),
                            xT_tiles[i][:, co, h * TCH:(h + 1) * TCH],
                            start=(co == 0),
                            stop=(co == CO - 1),
                        )
                A_sb[i] = abp.tile([P, T], bf16, tag="A", name=f"A{i}")
                # split cast: first half unblocks the si=0 score matmul
                nc.vector.tensor_copy(A_sb[i][:, 0:TCH], ps[:, 0:TCH])
                if i + 2 < BPC:
                    load_gated(i + 2, A_sb[i][:, 0:2])
                nc.vector.tensor_copy(A_sb[i][:, TCH:T], ps[:, TCH:T])

            def emit_projB(i, ps=None):
                if ps is None:
                    ps = ps_big.tile([P, 2 * TCH], f32, tag="ps",
                                     name=f"pB{i}")
                for h in range(2):
                    for co in range(CO):
                        nc.tensor.matmul(
                            ps[:, h * TCH:(h + 1) * TCH],
                            wB_co(co),
                            xT_tiles[i][:, co, h * TCH:(h + 1) * TCH],
                            start=(co == 0),
                            stop=(co == CO - 1),
                        )
                B_sb[i] = abp.tile([P, T], bf16, tag="B", name=f"B{i}")
                # split cast: the first 256 cols cover si=0/1; for item 0
                # they go on the still-idle scalar engine so the first
                # scores aren't serialized behind the DVE casts
                if i == 0:
                    nc.scalar.copy(B_sb[i][:, 0:P], ps[:, 0:P])
                    nc.scalar.copy(B_sb[i][:, P:2 * P], ps[:, P:2 * P])
                    nc.vector.tensor_copy(B_sb[i][:, 2 * P:T], ps[:, 2 * P:T])
                else:
                    nc.vector.tensor_copy(B_sb[i][:, 0:2 * P], ps[:, 0:2 * P])
                    nc.vector.tensor_copy(B_sb[i][:, 2 * P:T], ps[:, 2 * P:T])

            def emit_projV(i):
                psv = ps_sm.tile([P, NT, HS], f32, tag="sm", name=f"pV{i}")
                for ti in range(NT):
                    for co in range(CO):
                        nc.tensor.matmul(
                            psv[:, ti, :],
                            xT_tiles[i][:, co, ti * P:(ti + 1) * P],
                            wV_co(co),
                            start=(co == 0),
                            stop=(co == CO - 1),
                        )
                v = vaup.tile([P, NT, HS + 1], bf16, tag="vaug", name=f"va{i}")
                nc.gpsimd.memset(v[:, :, HS:HS + 1], 1.0)
                nc.vector.tensor_copy(v[:, :, 0:HS], psv)
                vaug[i] = v

            def emit_scores(i, grp):
                """matmul + exp + (pool) diagonal mask for one score group."""
                if grp < 4:
                    si = grp
                    t_lo = si * P
                    ncols = T - t_lo
                    ps = ps_big.tile([P, 2 * TCH], f32, tag="ps",
                                     name=f"sc{i}g{grp}")
                    for tj in range(2):
                        t0 = max(tj * TCH, t_lo)
                        t1 = (tj + 1) * TCH
                        if t0 >= t1:
                            continue
                        nc.tensor.matmul(
                            ps[:, t0:t1],
                            B_sb[i][0:HS, si * P:(si + 1) * P],
                            A_sb[i][0:HS, t0:t1],
                            start=True,
                            stop=True,
                        )
                    ew = expwp.tile([P, ncols], bf16, tag=f"ew{grp}",
                                    name=f"ew{i}g{grp}")
                    nc.scalar.activation(ew, ps[:, t_lo:T], Exp, scale=0.125)
                    nc.gpsimd.tensor_mul(ew[:, 0:P], ew[:, 0:P], mask_sb)
                elif grp == 4:
                    # blocks: si4 at 0:4 (t = 512 + 128b), si5 at 4:7
                    # (t = 640 + 128(b-4)); no garbage block.
                    ps = ps_big.tile([P, 7, P], f32, tag="ps",
                                     name=f"sc{i}g4")
                    nc.tensor.matmul(
                        ps[:, 0:4, :],
                        B_sb[i][0:HS, 4 * P:5 * P],
                        A_sb[i][0:HS, TCH:T],
                        start=True,
                        stop=True,
                    )
                    nc.tensor.matmul(
                        ps[:, 4:7, :],
                        B_sb[i][0:HS, 5 * P:6 * P],
                        A_sb[i][0:HS, 5 * P:T],
                        start=True,
                        stop=True,
                    )
                    ew = expwp.tile([P, 7, P], bf16, tag="ew4",
                                    name=f"ew{i}g4")
                    nc.scalar.activation(ew, ps, Exp, scale=0.125)
                    # diagonals: si4 at block 0, si5 at block 4
                    nc.gpsimd.tensor_mul(
                        ew[:, 0:5:4, :],
                        ew[:, 0:5:4, :],
                        mask_sb[:, None, :].to_broadcast([P, 2, P]),
                    )
                else:
                    # blocks: si6 at 0:2 (t = 768 + 128b), si7 at 2 (t=896+)
                    ps = ps_big.tile([P, 3, P], f32, tag="ps",
                                     name=f"sc{i}g5")
                    nc.tensor.matmul(
                        ps[:, 0:2, :],
                        B_sb[i][0:HS, 6 * P:7 * P],
                        A_sb[i][0:HS, 6 * P:T],
                        start=True,
                        stop=True,
                    )
                    nc.tensor.matmul(
                        ps[:, 2, :],
                        B_sb[i][0:HS, 7 * P:T],
                        A_sb[i][0:HS, 7 * P:T],
                        start=True,
                        stop=True,
                    )
                    ew = expwp.tile([P, 3, P], bf16, tag="ew5",
                                    name=f"ew{i}g5")
                    nc.scalar.activation(ew, ps, Exp, scale=0.125)
                    # diagonals: si6 at block 0, si7 at block 2
                    nc.gpsimd.tensor_mul(
                        ew[:, 0:3:2, :],
                        ew[:, 0:3:2, :],
                        mask_sb[:, None, :].to_broadcast([P, 2, P]),
                    )
                expw[(i, grp)] = ew

            def ew_chunk(i, si, ti):
                """128-wide ew column chunk for (si, ti)."""
                if si < 4:
                    ew = expw[(i, si)]
                    c0 = (ti - si) * P
                    return ew[:, c0:c0 + P]
                if si == 4:
                    return expw[(i, 4)][:, ti - 4, :]
                if si == 5:
                    return expw[(i, 4)][:, ti - 1, :]
                if si == 6:
                    return expw[(i, 5)][:, ti - 6, :]
                return expw[(i, 5)][:, 2, :]

            def emit_pv(i, tis):
                for ti in tis:
                    half = ti // 4
                    tii = ti % 4
                    if (i, half) not in po:
                        po[(i, half)] = ps_sm.tile(
                            [P, 4, HS + 1], f32, tag="sm", name=f"po{i}h{half}"
                        )
                    p = po[(i, half)]
                    for si in range(ti + 1):
                        nc.tensor.matmul(
                            p[:, tii, :],
                            ew_chunk(i, si, ti),
                            vaug[i][:, si, :],
                            start=(si == 0),
                            stop=(si == ti),
                        )

            def emit_norm_out(i, half):
                p = po[(i, half)]
                o = outp.tile([P, 4, HS], f32, tag="osb", name=f"o{i}h{half}")
                r = outp.tile([P, 4], f32, tag="recip", name=f"r{i}h{half}")
                nc.vector.reciprocal(r, p[:, :, HS])
                nc.vector.tensor_tensor(
                    o, p[:, :, 0:HS],
                    r[:, :, None].to_broadcast([P, 4, HS]),
                    mybir.AluOpType.mult,
                )
                osb[(i, half)] = o
                dst = out[i].rearrange("(ti p) h -> p ti h", p=P)
                nc.sync.dma_start(dst[:, half * 4:(half + 1) * 4, :], o)

            # ---- software-pipelined emission ----------------------------
            # PE order is chosen so the scalar engine (exp; the pacing
            # engine) never waits: score groups are back-to-back, the next
            # item's first two score groups are emitted right after this
            # item's last, and PV / V-projection fill the PE slack.
            emit_projA(0)
            emit_projB(0)
            emit_scores(0, 0)
            emit_scores(0, 1)
            emit_scores(0, 2)
            emit_scores(0, 3)
            emit_projA(1)
            emit_scores(0, 4)
            emit_projB(1)
            emit_scores(0, 5)
            for i in range(BPC):
                # entering here: scores(i, 0..5) all emitted; A/B(i+1)
                # emitted; PV(i), V(i), norms(i), scores(i+1, *),
                # A/B(i+2) still to do.
                nxt = i + 1 < BPC
                if nxt:
                    emit_scores(i + 1, 0)
                    emit_scores(i + 1, 1)
                emit_projV(i)
                emit_pv(i, [0, 1, 2, 3])
                emit_norm_out(i, 0)
                emit_pv(i, [4, 5])
                if nxt:
                    emit_scores(i + 1, 2)
                if i + 2 < BPC:
                    emit_projA(i + 2)
                if nxt:
                    emit_scores(i + 1, 3)
                emit_pv(i, [6, 7])
                emit_norm_out(i, 1)
                if nxt:
                    emit_scores(i + 1, 4)
                    emit_scores(i + 1, 5)
                if i + 2 < BPC:
                    emit_projB(i + 2)

    nc.compile()
    return nc


def _get_nc():
    nc = _cached.get("nc")
    if nc is None:
        nc = _build()
        _cached["nc"] = nc
    return nc


def _in_maps(x, Wk, Wq, Wv):
    bf = ml_dtypes.bfloat16
    x = np.asarray(x, dtype=np.float32)
    Wk = np.asarray(Wk, dtype=np.float32)
    Wq = np.asarray(Wq, dtype=np.float32)
    Wv = np.asarray(Wv, dtype=np.float32)
    # packed per-partition weight blob [P, 768]:
    # [wA(co0)|wA(co1)|wB(co0)|wB(co1)|wV(co0)|wV(co1)|mask]
    # where wX(co) is W{X}.T[co*128:(co+1)*128, :] laid out so partition p
    # holds contraction row p of chunk co.
    wA = np.concatenate([Wq.T, Wk.T], axis=1)   # [C, 2HS]
    wB = np.concatenate([Wk.T, Wq.T], axis=1)   # [C, 2HS]
    wV = Wv.T                                   # [C, HS]
    m = np.triu(np.ones((P, P), dtype=np.float32))
    wcat = np.concatenate(
        [
            wA[0:P, :], wA[P:C, :],
            wB[0:P, :], wB[P:C, :],
            wV[0:P, :], wV[P:C, :],
            m,
        ],
        axis=1,
    )
    wcat = np.ascontiguousarray(wcat).astype(bf)
    maps = []
    for c in range(NCORES):
        xs = x[c * BPC:(c + 1) * BPC]
        xsT = np.ascontiguousarray(xs.transpose(0, 2, 1)).astype(bf)
        maps.append({"xT": xsT, "wcat": wcat})
    return maps


def _run(x, Wk, Wq, Wv, **spmd_kwargs):
    from concourse.bass_utils import run_bass_kernel_spmd

    nc = _get_nc()
    res = run_bass_kernel_spmd(
        nc, _in_maps(x, Wk, Wq, Wv), core_ids=list(range(NCORES)), **spmd_kwargs
    )
    full = np.concatenate([r["out"] for r in res.results], axis=0)
    return full, res


def kernel(x, Wk, Wq, Wv):
    full, _ = _run(x, Wk, Wq, Wv)
    return full


# revision 8
# speedup vs baseline: 1.0546x; 1.0546x over previous
"""Causal single-head attention on 8 Trainium2 NeuronCores (batch-parallel), v2.

Problem (nn_Head): x[32,1024,256] f32, Wk/Wq/Wv[64,256] f32.
  q/k/v = x @ W.T ; wei = softmax(causal(q @ k.T / 8)) ; out = wei @ v.

Sharding: B=32 split 4-per-core across 8 cores; weights replicated.

v2 changes vs baseline:
  - weights DMA first on the scalar queue (no longer queued behind x).
  - all items' x loads issued upfront (co0 on sync, co1 on gpsimd queues).
  - PE warmup dummy matmuls so real matmuls run at 2.4 GHz (HAM warm).
  - scores for si=4,5 share one PSUM tile (one exp), si=6,7 share one bank
    (one exp): 6 ACTIVATEs/item instead of 8 (ScalarE is the pacing engine).
  - diagonal mask-multiplies moved to the idle Pool (gpsimd) engine.
  - explicit software pipeline: next item's projections are emitted between
    this item's score matmuls; next item's first score matmul is emitted
    before this item's PV tail so ScalarE never starves.
  - outputs stored per-half, all on the sync queue (inputs done by then).
"""

import numpy as np
import ml_dtypes

B, T, C, HS = 32, 1024, 256, 64
NCORES = 8
BPC = B // NCORES  # batch items per core
P = 128            # partitions / row-tile
NT = T // P        # 8 row tiles per item
CO = C // P        # 2 contraction chunks for projections
TCH = 512          # matmul free-dim chunk (one PSUM bank of f32)
N_WARMUP = 5       # dummy matmuls (N=512) to warm the PE clock

_cached = {}


def _build():
    import concourse.tile as tile
    from concourse import bacc, mybir

    bf16 = mybir.dt.bfloat16
    f32 = mybir.dt.float32
    Exp = mybir.ActivationFunctionType.Exp
    Mult = mybir.AluOpType.mult

    nc = bacc.Bacc(
        "TRN2",
        target_bir_lowering=False,
        debug=False,
        num_devices=NCORES,
    )

    xT = nc.dram_tensor("xT", [BPC, C, T], bf16, kind="ExternalInput")
    # packed weights, one DMA: per partition p the 768 bf16 columns are
    # [wA(co0)|wA(co1)|wB(co0)|wB(co1)|wV(co0)|wV(co1)|mask]
    wcat = nc.dram_tensor("wcat", [P, 768], bf16, kind="ExternalInput")
    out = nc.dram_tensor("out", [BPC, T, HS], f32, kind="ExternalOutput")

    # score-group layout: group id -> (si list, psum cols, act window)
    #   groups 0..3 hold a single si in a [P, 1024] tile (2 banks), valid
    #   t from 128*si, exp reads exactly the causal span.
    #   group 4 = {si4, si5} in [P, 1024]: si4 at cols 0:512 (t=512..1024),
    #   si5 at cols 512:1024 (same t window; valid from t=640) -> one exp
    #   of 1024 cols (128 garbage, never read downstream).
    #   group 5 = {si6, si7} in [P, 512] (1 bank): si6 at 0:256
    #   (t=768..1024), si7 at 256:384 (t=896..1024) -> one exp of 384 cols.

    with tile.TileContext(nc) as tc:
        with (
            tc.tile_pool(name="consts", bufs=1) as consts,
            tc.tile_pool(name="xin", bufs=4) as xin,
            tc.tile_pool(name="ab", bufs=4) as abp,
            tc.tile_pool(name="vau", bufs=3) as vaup,
            tc.tile_pool(name="expw", bufs=2) as expwp,
            tc.tile_pool(name="outp", bufs=3) as outp,
            tc.tile_pool(name="ps_big", bufs=3, space="PSUM") as ps_big,
            tc.tile_pool(name="ps_sm", bufs=2, space="PSUM") as ps_sm,
        ):
            # ---- input DMAs ---------------------------------------------
            # packed weights blob: one trigger, first thing on the sync
            # queue so it lands before the (larger) x pieces behind it.
            wcat_sb = consts.tile([P, 768], bf16, tag="wcat")
            nc.sync.dma_start(wcat_sb, wcat[:, :])

            def wA_co(co):
                return wcat_sb[:, co * P:(co + 1) * P]

            def wB_co(co):
                return wcat_sb[:, 256 + co * P:256 + (co + 1) * P]

            def wV_co(co):
                return wcat_sb[:, 512 + co * HS:512 + (co + 1) * HS]

            mask_sb = consts.tile([P, P], bf16, tag="mask")
            nc.gpsimd.tensor_copy(mask_sb, wcat_sb[:, 640:768])

            # warmup source first so the gpsimd memset isn't queued behind
            # the DMA trigger instructions
            dummy_src = consts.tile([P, TCH], bf16, tag="dummy")
            nc.gpsimd.memset(dummy_src, 0.0)

            # x loads: co0 on the sync HW queue, co1 on the scalar HW
            # queue.  The DMA engines round-robin across every queued
            # transfer, so item 0 (t-halved, 4 pieces) goes up alone;
            # each later item's DMA is WAW-gated on a tiny copy that
            # depends on the PREVIOUS item's A-cast, which serializes
            # the items on the wire in the order compute needs them.
            xT_tiles = []
            gate_src = {}
            for it in range(BPC):
                t = xin.tile([P, CO, T], bf16, tag="xT", name=f"xT{it}")
                r = xT[it].rearrange("(co p) t -> p co t", p=P)
                xT_tiles.append(t)
            for h in range(2):
                nc.sync.dma_start(
                    xT_tiles[0][:, 0:1, h * TCH:(h + 1) * TCH],
                    xT[0].rearrange("(co p) t -> p co t", p=P)[
                        :, 0:1, h * TCH:(h + 1) * TCH],
                )
                nc.scalar.dma_start(
                    xT_tiles[0][:, 1:2, h * TCH:(h + 1) * TCH],
                    xT[0].rearrange("(co p) t -> p co t", p=P)[
                        :, 1:2, h * TCH:(h + 1) * TCH],
                )

            def load_gated(it, gate_ap, co1_eng=None):
                # items 2/3 go sync-only: a late-gated trigger in the
                # scalar engine's stream would block the exp pipeline
                # (in-order engine).  item 1's gate fires before the
                # first exp, so its co1 half may use the scalar queue.
                t = xT_tiles[it]
                r = xT[it].rearrange("(co p) t -> p co t", p=P)
                nc.gpsimd.tensor_copy(t[:, 0, 0:2], gate_ap)
                nc.gpsimd.tensor_copy(t[:, 1, 0:2], gate_ap)
                nc.sync.dma_start(t[:, 0:1, :], r[:, 0:1, :])
                (co1_eng or nc.sync).dma_start(t[:, 1:2, :], r[:, 1:2, :])

            # item 1: gated on item 0's own co1 data having landed.
            # co1 stays on sync too: a gated trigger at the top of the
            # scalar stream would delay the ACT table load behind it.
            load_gated(1, xT_tiles[0][:, 1, T - 2:T])

            # ---- PE warmup ----------------------------------------------
            ps_warm = ps_big.tile([P, 2, TCH], f32, tag="ps", name="warm")
            for w in range(N_WARMUP):
                nc.tensor.matmul(
                    ps_warm[:, w % 2, :],
                    dummy_src[:, 0:P],
                    dummy_src,
                    start=True,
                    stop=True,
                )

            # ---- per-item emitters --------------------------------------
            A_sb = {}
            B_sb = {}
            vaug = {}
            expw = {}   # (item, grp) -> ew tile
            po = {}     # (item, half) -> psum tile
            osb = {}

            def emit_projA(i, ps=None):
                if ps is None:
                    ps = ps_big.tile([P, 2 * TCH], f32, tag="ps",
                                     name=f"pA{i}")
                for h in range(2):
                    for co in range(CO):
                        nc.tensor.matmul(
                            ps[:, h * TCH:(h + 1) * TCH],
                            wA_co(co),
                            xT_tiles[i][:, co, h * TCH:(h + 1) * TCH],
                            start=(co == 0),
                            stop=(co == CO - 1),
                        )
                A_sb[i] = abp.tile([P, T], bf16, tag="A", name=f"A{i}")
                # split cast: first half unblocks the si=0 score matmul
                nc.vector.tensor_copy(A_sb[i][:, 0:TCH], ps[:, 0:TCH])
                if i + 2 < BPC:
                    load_gated(i + 2, A_sb[i][:, 0:2])
                nc.vector.tensor_copy(A_sb[i][:, TCH:T], ps[:, TCH:T])

            def emit_projB(i, ps=None):
                if ps is None:
                    ps = ps_big.tile([P, 2 * TCH], f32, tag="ps",
                                     name=f"pB{i}")
                for h in range(2):
                    for co in range(CO):
                        nc.tensor.matmul(
                            ps[:, h * TCH:(h + 1) * TCH],
                            wB_co(co),
                            xT_tiles[i][:, co, h * TCH:(h + 1) * TCH],
                            start=(co == 0),
                            stop=(co == CO - 1),
                        )
                B_sb[i] = abp.tile([P, T], bf16, tag="B", name=f"B{i}")
                # split cast: the first 256 cols cover si=0/1; for item 0
                # they go on the still-idle scalar engine so the first
                # scores aren't serialized behind the DVE casts
                if i == 0:
                    nc.scalar.copy(B_sb[i][:, 0:P], ps[:, 0:P])
                    nc.scalar.copy(B_sb[i][:, P:2 * P], ps[:, P:2 * P])
                    nc.vector.tensor_copy(B_sb[i][:, 2 * P:T], ps[:, 2 * P:T])
                else:
                    nc.vector.tensor_copy(B_sb[i][:, 0:2 * P], ps[:, 0:2 * P])
                    nc.vector.tensor_copy(B_sb[i][:, 2 * P:T], ps[:, 2 * P:T])

            def emit_projV(i):
                psv = ps_sm.tile([P, NT, HS], f32, tag="sm", name=f"pV{i}")
                for ti in range(NT):
                    for co in range(CO):
                        nc.tensor.matmul(
                            psv[:, ti, :],
                            xT_tiles[i][:, co, ti * P:(ti + 1) * P],
                            wV_co(co),
                            start=(co == 0),
                            stop=(co == CO - 1),
                        )
                v = vaup.tile([P, NT, HS + 1], bf16, tag="vaug", name=f"va{i}")
                nc.gpsimd.memset(v[:, :, HS:HS + 1], 1.0)
                nc.vector.tensor_copy(v[:, :, 0:HS], psv)
                vaug[i] = v

            def emit_scores(i, grp):
                """matmul + exp + (pool) diagonal mask for one score group."""
                if grp < 4:
                    si = grp
                    t_lo = si * P
                    ncols = T - t_lo
                    ps = ps_big.tile([P, 2 * TCH], f32, tag="ps",
                                     name=f"sc{i}g{grp}")
                    for tj in range(2):
                        t0 = max(tj * TCH, t_lo)
                        t1 = (tj + 1) * TCH
                        if t0 >= t1:
                            continue
                        nc.tensor.matmul(
                            ps[:, t0:t1],
                            B_sb[i][0:HS, si * P:(si + 1) * P],
                            A_sb[i][0:HS, t0:t1],
                            start=True,
                            stop=True,
                        )
                    ew = expwp.tile([P, ncols], bf16, tag=f"ew{grp}",
                                    name=f"ew{i}g{grp}")
                    nc.scalar.activation(ew, ps[:, t_lo:T], Exp, scale=0.125)
                    nc.gpsimd.tensor_mul(ew[:, 0:P], ew[:, 0:P], mask_sb)
                elif grp == 4:
                    # blocks: si4 at 0:4 (t = 512 + 128b), si5 at 4:7
                    # (t = 640 + 128(b-4)); no garbage block.
                    ps = ps_big.tile([P, 7, P], f32, tag="ps",
                                     name=f"sc{i}g4")
                    nc.tensor.matmul(
                        ps[:, 0:4, :],
                        B_sb[i][0:HS, 4 * P:5 * P],
                        A_sb[i][0:HS, TCH:T],
                        start=True,
                        stop=True,
                    )
                    nc.tensor.matmul(
                        ps[:, 4:7, :],
                        B_sb[i][0:HS, 5 * P:6 * P],
                        A_sb[i][0:HS, 5 * P:T],
                        start=True,
                        stop=True,
                    )
                    ew = expwp.tile([P, 7, P], bf16, tag="ew4",
                                    name=f"ew{i}g4")
                    nc.scalar.activation(ew, ps, Exp, scale=0.125)
                    # diagonals: si4 at block 0, si5 at block 4
                    nc.gpsimd.tensor_mul(
                        ew[:, 0:5:4, :],
                        ew[:, 0:5:4, :],
                        mask_sb[:, None, :].to_broadcast([P, 2, P]),
                    )
                else:
                    # blocks: si6 at 0:2 (t = 768 + 128b), si7 at 2 (t=896+)
                    ps = ps_big.tile([P, 3, P], f32, tag="ps",
                                     name=f"sc{i}g5")
                    nc.tensor.matmul(
                        ps[:, 0:2, :],
                        B_sb[i][0:HS, 6 * P:7 * P],
                        A_sb[i][0:HS, 6 * P:T],
                        start=True,
                        stop=True,
                    )
                    nc.tensor.matmul(
                        ps[:, 2, :],
                        B_sb[i][0:HS, 7 * P:T],
                        A_sb[i][0:HS, 7 * P:T],
                        start=True,
                        stop=True,
                    )
                    ew = expwp.tile([P, 3, P], bf16, tag="ew5",
                                    name=f"ew{i}g5")
                    nc.scalar.activation(ew, ps, Exp, scale=0.125)
                    # diagonals: si6 at block 0, si7 at block 2
                    nc.gpsimd.tensor_mul(
                        ew[:, 0:3:2, :],
                        ew[:, 0:3:2, :],
                        mask_sb[:, None, :].to_broadcast([P, 2, P]),
                    )
                expw[(i, grp)] = ew

            def ew_chunk(i, si, ti):
                """128-wide ew column chunk for (si, ti)."""
                if si < 4:
                    ew = expw[(i, si)]
                    c0 = (ti - si) * P
                    return ew[:, c0:c0 + P]
                if si == 4:
                    return expw[(i, 4)][:, ti - 4, :]
                if si == 5:
                    return expw[(i, 4)][:, ti - 1, :]
                if si == 6:
                    return expw[(i, 5)][:, ti - 6, :]
                return expw[(i, 5)][:, 2, :]

            def emit_pv(i, tis):
                for ti in tis:
                    half = ti // 4
                    tii = ti % 4
                    if (i, half) not in po:
                        po[(i, half)] = ps_sm.tile(
                            [P, 4, HS + 1], f32, tag="sm", name=f"po{i}h{half}"
                        )
                    p = po[(i, half)]
                    for si in range(ti + 1):
                        nc.tensor.matmul(
                            p[:, tii, :],
                            ew_chunk(i, si, ti),
                            vaug[i][:, si, :],
                            start=(si == 0),
                            stop=(si == ti),
                        )

            def emit_norm_out(i, half):
                p = po[(i, half)]
                o = outp.tile([P, 4, HS], f32, tag="osb", name=f"o{i}h{half}")
                r = outp.tile([P, 4], f32, tag="recip", name=f"r{i}h{half}")
                nc.vector.reciprocal(r, p[:, :, HS])
                nc.vector.tensor_tensor(
                    o, p[:, :, 0:HS],
                    r[:, :, None].to_broadcast([P, 4, HS]),
                    mybir.AluOpType.mult,
                )
                osb[(i, half)] = o
                dst = out[i].rearrange("(ti p) h -> p ti h", p=P)
                nc.sync.dma_start(dst[:, half * 4:(half + 1) * 4, :], o)

            # ---- software-pipelined emission ----------------------------
            # PE order is chosen so the scalar engine (exp; the pacing
            # engine) never waits: score groups are back-to-back, the next
            # item's first two score groups are emitted right after this
            # item's last, and PV / V-projection fill the PE slack.
            emit_projA(0)
            emit_projB(0)
            emit_scores(0, 0)
            emit_scores(0, 1)
            emit_scores(0, 2)
            emit_scores(0, 3)
            emit_projA(1)
            emit_scores(0, 4)
            emit_projB(1)
            emit_scores(0, 5)
            for i in range(BPC):
                # entering here: scores(i, 0..5) all emitted; A/B(i+1)
                # emitted; PV(i), V(i), norms(i), scores(i+1, *),
                # A/B(i+2) still to do.
                nxt = i + 1 < BPC
                if nxt:
                    emit_scores(i + 1, 0)
                    emit_scores(i + 1, 1)
                emit_projV(i)
                emit_pv(i, [0, 1, 2, 3])
                emit_norm_out(i, 0)
                emit_pv(i, [4, 5])
                if nxt:
                    emit_scores(i + 1, 2)
                if i + 2 < BPC:
                    emit_projA(i + 2)
                if nxt:
                    emit_scores(i + 1, 3)
                if i + 2 < BPC:
                    emit_projB(i + 2)
                emit_pv(i, [6, 7])
                emit_norm_out(i, 1)
                if nxt:
                    emit_scores(i + 1, 4)
                    emit_scores(i + 1, 5)

    nc.compile()
    return nc


def _get_nc():
    nc = _cached.get("nc")
    if nc is None:
        nc = _build()
        _cached["nc"] = nc
    return nc


def _in_maps(x, Wk, Wq, Wv):
    bf = ml_dtypes.bfloat16
    x = np.asarray(x, dtype=np.float32)
    Wk = np.asarray(Wk, dtype=np.float32)
    Wq = np.asarray(Wq, dtype=np.float32)
    Wv = np.asarray(Wv, dtype=np.float32)
    # packed per-partition weight blob [P, 768]:
    # [wA(co0)|wA(co1)|wB(co0)|wB(co1)|wV(co0)|wV(co1)|mask]
    # where wX(co) is W{X}.T[co*128:(co+1)*128, :] laid out so partition p
    # holds contraction row p of chunk co.
    wA = np.concatenate([Wq.T, Wk.T], axis=1)   # [C, 2HS]
    wB = np.concatenate([Wk.T, Wq.T], axis=1)   # [C, 2HS]
    wV = Wv.T                                   # [C, HS]
    m = np.triu(np.ones((P, P), dtype=np.float32))
    wcat = np.concatenate(
        [
            wA[0:P, :], wA[P:C, :],
            wB[0:P, :], wB[P:C, :],
            wV[0:P, :], wV[P:C, :],
            m,
        ],
        axis=1,
    )
    wcat = np.ascontiguousarray(wcat).astype(bf)
    maps = []
    for c in range(NCORES):
        xs = x[c * BPC:(c + 1) * BPC]
        xsT = np.ascontiguousarray(xs.transpose(0, 2, 1)).astype(bf)
        maps.append({"xT": xsT, "wcat": wcat})
    return maps


def _run(x, Wk, Wq, Wv, **spmd_kwargs):
    from concourse.bass_utils import run_bass_kernel_spmd

    nc = _get_nc()
    res = run_bass_kernel_spmd(
        nc, _in_maps(x, Wk, Wq, Wv), core_ids=list(range(NCORES)), **spmd_kwargs
    )
    full = np.concatenate([r["out"] for r in res.results], axis=0)
    return full, res


def kernel(x, Wk, Wq, Wv):
    full, _ = _run(x, Wk, Wq, Wv)
    return full


# revision 9
# speedup vs baseline: 1.1373x; 1.0784x over previous
"""Causal single-head attention on 8 Trainium2 NeuronCores (batch-parallel), v2.

Problem (nn_Head): x[32,1024,256] f32, Wk/Wq/Wv[64,256] f32.
  q/k/v = x @ W.T ; wei = softmax(causal(q @ k.T / 8)) ; out = wei @ v.

Sharding: B=32 split 4-per-core across 8 cores; weights replicated.

v2 changes vs baseline:
  - weights DMA first on the scalar queue (no longer queued behind x).
  - all items' x loads issued upfront (co0 on sync, co1 on gpsimd queues).
  - PE warmup dummy matmuls so real matmuls run at 2.4 GHz (HAM warm).
  - scores for si=4,5 share one PSUM tile (one exp), si=6,7 share one bank
    (one exp): 6 ACTIVATEs/item instead of 8 (ScalarE is the pacing engine).
  - diagonal mask-multiplies moved to the idle Pool (gpsimd) engine.
  - explicit software pipeline: next item's projections are emitted between
    this item's score matmuls; next item's first score matmul is emitted
    before this item's PV tail so ScalarE never starves.
  - outputs stored per-half, all on the sync queue (inputs done by then).
"""

import numpy as np
import ml_dtypes

B, T, C, HS = 32, 1024, 256, 64
NCORES = 8
BPC = B // NCORES  # batch items per core
P = 128            # partitions / row-tile
NT = T // P        # 8 row tiles per item
CO = C // P        # 2 contraction chunks for projections
TCH = 512          # matmul free-dim chunk (one PSUM bank of f32)
N_WARMUP = 5       # dummy matmuls (N=512) to warm the PE clock

_cached = {}


def _build():
    import concourse.tile as tile
    from concourse import bacc, mybir

    bf16 = mybir.dt.bfloat16
    f32 = mybir.dt.float32
    Exp = mybir.ActivationFunctionType.Exp
    Mult = mybir.AluOpType.mult

    nc = bacc.Bacc(
        "TRN2",
        target_bir_lowering=False,
        debug=False,
        num_devices=NCORES,
    )

    xT = nc.dram_tensor("xT", [BPC, C, T], bf16, kind="ExternalInput")
    # packed weights, one DMA: per partition p the 768 bf16 columns are
    # [wA(co0)|wA(co1)|wB(co0)|wB(co1)|wV(co0)|wV(co1)|mask]
    wcat = nc.dram_tensor("wcat", [P, 768], bf16, kind="ExternalInput")
    out = nc.dram_tensor("out", [BPC, T, HS], f32, kind="ExternalOutput")

    # score-group layout: group id -> (si list, psum cols, act window)
    #   groups 0..3 hold a single si in a [P, 1024] tile (2 banks), valid
    #   t from 128*si, exp reads exactly the causal span.
    #   group 4 = {si4, si5} in [P, 1024]: si4 at cols 0:512 (t=512..1024),
    #   si5 at cols 512:1024 (same t window; valid from t=640) -> one exp
    #   of 1024 cols (128 garbage, never read downstream).
    #   group 5 = {si6, si7} in [P, 512] (1 bank): si6 at 0:256
    #   (t=768..1024), si7 at 256:384 (t=896..1024) -> one exp of 384 cols.

    with tile.TileContext(nc) as tc:
        with (
            tc.tile_pool(name="consts", bufs=1) as consts,
            tc.tile_pool(name="xin", bufs=4) as xin,
            tc.tile_pool(name="ab", bufs=4) as abp,
            tc.tile_pool(name="vau", bufs=3) as vaup,
            tc.tile_pool(name="expw", bufs=2) as expwp,
            tc.tile_pool(name="outp", bufs=3) as outp,
            tc.tile_pool(name="ps_big", bufs=3, space="PSUM") as ps_big,
            tc.tile_pool(name="ps_sm", bufs=2, space="PSUM") as ps_sm,
        ):
            # ---- input DMAs ---------------------------------------------
            # packed weights blob: one trigger, first thing on the sync
            # queue so it lands before the (larger) x pieces behind it.
            wcat_sb = consts.tile([P, 768], bf16, tag="wcat")
            nc.sync.dma_start(wcat_sb, wcat[:, :])

            def wA_co(co):
                return wcat_sb[:, co * P:(co + 1) * P]

            def wB_co(co):
                return wcat_sb[:, 256 + co * P:256 + (co + 1) * P]

            def wV_co(co):
                return wcat_sb[:, 512 + co * HS:512 + (co + 1) * HS]

            mask_sb = consts.tile([P, P], bf16, tag="mask")
            nc.gpsimd.tensor_copy(mask_sb, wcat_sb[:, 640:768])

            # warmup source first so the gpsimd memset isn't queued behind
            # the DMA trigger instructions
            dummy_src = consts.tile([P, TCH], bf16, tag="dummy")
            nc.gpsimd.memset(dummy_src, 0.0)

            # x loads: co0 on the sync HW queue, co1 on the scalar HW
            # queue.  The DMA engines round-robin across every queued
            # transfer, so item 0 (t-halved, 4 pieces) goes up alone;
            # each later item's DMA is WAW-gated on a tiny copy that
            # depends on the PREVIOUS item's A-cast, which serializes
            # the items on the wire in the order compute needs them.
            xT_tiles = []
            gate_src = {}
            for it in range(BPC):
                t = xin.tile([P, CO, T], bf16, tag="xT", name=f"xT{it}")
                r = xT[it].rearrange("(co p) t -> p co t", p=P)
                xT_tiles.append(t)
            for h in range(2):
                nc.sync.dma_start(
                    xT_tiles[0][:, 0:1, h * TCH:(h + 1) * TCH],
                    xT[0].rearrange("(co p) t -> p co t", p=P)[
                        :, 0:1, h * TCH:(h + 1) * TCH],
                )
                nc.scalar.dma_start(
                    xT_tiles[0][:, 1:2, h * TCH:(h + 1) * TCH],
                    xT[0].rearrange("(co p) t -> p co t", p=P)[
                        :, 1:2, h * TCH:(h + 1) * TCH],
                )

            def load_gated(it, gate_ap, co1_eng=None):
                # items 2/3 go sync-only: a late-gated trigger in the
                # scalar engine's stream would block the exp pipeline
                # (in-order engine).  item 1's gate fires before the
                # first exp, so its co1 half may use the scalar queue.
                t = xT_tiles[it]
                r = xT[it].rearrange("(co p) t -> p co t", p=P)
                nc.gpsimd.tensor_copy(t[:, 0, 0:2], gate_ap)
                nc.gpsimd.tensor_copy(t[:, 1, 0:2], gate_ap)
                nc.sync.dma_start(t[:, 0:1, :], r[:, 0:1, :])
                (co1_eng or nc.sync).dma_start(t[:, 1:2, :], r[:, 1:2, :])

            # item 1: gated on item 0's own co1 data having landed.
            # co1 stays on sync too: a gated trigger at the top of the
            # scalar stream would delay the ACT table load behind it.
            load_gated(1, xT_tiles[0][:, 1, T - 2:T])

            # ---- PE warmup ----------------------------------------------
            ps_warm = ps_big.tile([P, 2, TCH], f32, tag="ps", name="warm")
            for w in range(N_WARMUP):
                nc.tensor.matmul(
                    ps_warm[:, w % 2, :],
                    dummy_src[:, 0:P],
                    dummy_src,
                    start=True,
                    stop=True,
                )

            # ---- per-item emitters --------------------------------------
            A_sb = {}
            B_sb = {}
            vaug = {}
            expw = {}   # (item, grp) -> ew tile
            po = {}     # (item, half) -> psum tile
            osb = {}

            def emit_projA(i, ps=None):
                if ps is None:
                    ps = ps_big.tile([P, 2 * TCH], f32, tag="ps",
                                     name=f"pA{i}")
                for h in range(2):
                    for co in range(CO):
                        nc.tensor.matmul(
                            ps[:, h * TCH:(h + 1) * TCH],
                            wA_co(co),
                            xT_tiles[i][:, co, h * TCH:(h + 1) * TCH],
                            start=(co == 0),
                            stop=(co == CO - 1),
                        )
                A_sb[i] = abp.tile([P, T], bf16, tag="A", name=f"A{i}")
                # split cast: first half unblocks the si=0 score matmul
                nc.vector.tensor_copy(A_sb[i][:, 0:TCH], ps[:, 0:TCH])
                if i + 2 < BPC:
                    load_gated(i + 2, A_sb[i][:, 0:2])
                nc.vector.tensor_copy(A_sb[i][:, TCH:T], ps[:, TCH:T])

            def emit_projB(i, ps=None):
                if ps is None:
                    ps = ps_big.tile([P, 2 * TCH], f32, tag="ps",
                                     name=f"pB{i}")
                for h in range(2):
                    for co in range(CO):
                        nc.tensor.matmul(
                            ps[:, h * TCH:(h + 1) * TCH],
                            wB_co(co),
                            xT_tiles[i][:, co, h * TCH:(h + 1) * TCH],
                            start=(co == 0),
                            stop=(co == CO - 1),
                        )
                B_sb[i] = abp.tile([P, T], bf16, tag="B", name=f"B{i}")
                # split cast: the first 256 cols cover si=0/1; for item 0
                # they go on the still-idle scalar engine so the first
                # scores aren't serialized behind the DVE casts
                if i == 0:
                    nc.scalar.copy(B_sb[i][:, 0:P], ps[:, 0:P])
                    nc.scalar.copy(B_sb[i][:, P:2 * P], ps[:, P:2 * P])
                    nc.vector.tensor_copy(B_sb[i][:, 2 * P:T], ps[:, 2 * P:T])
                else:
                    nc.vector.tensor_copy(B_sb[i][:, 0:2 * P], ps[:, 0:2 * P])
                    nc.vector.tensor_copy(B_sb[i][:, 2 * P:T], ps[:, 2 * P:T])

            def emit_projV(i):
                psv = ps_sm.tile([P, NT, HS], f32, tag="sm", name=f"pV{i}")
                for ti in range(NT):
                    for co in range(CO):
                        nc.tensor.matmul(
                            psv[:, ti, :],
                            xT_tiles[i][:, co, ti * P:(ti + 1) * P],
                            wV_co(co),
                            start=(co == 0),
                            stop=(co == CO - 1),
                        )
                v = vaup.tile([P, NT, HS + 1], bf16, tag="vaug", name=f"va{i}")
                nc.gpsimd.memset(v[:, :, HS:HS + 1], 1.0)
                nc.vector.tensor_copy(v[:, :, 0:HS], psv)
                vaug[i] = v

            def emit_scores(i, grp):
                """matmul + exp + (pool) diagonal mask for one score group."""
                if grp < 4:
                    si = grp
                    t_lo = si * P
                    ncols = T - t_lo
                    ps = ps_big.tile([P, 2 * TCH], f32, tag="ps",
                                     name=f"sc{i}g{grp}")
                    for tj in range(2):
                        t0 = max(tj * TCH, t_lo)
                        t1 = (tj + 1) * TCH
                        if t0 >= t1:
                            continue
                        nc.tensor.matmul(
                            ps[:, t0:t1],
                            B_sb[i][0:HS, si * P:(si + 1) * P],
                            A_sb[i][0:HS, t0:t1],
                            start=True,
                            stop=True,
                        )
                    ew = expwp.tile([P, ncols], bf16, tag=f"ew{grp}",
                                    name=f"ew{i}g{grp}")
                    nc.scalar.activation(ew, ps[:, t_lo:T], Exp, scale=0.125)
                    nc.gpsimd.tensor_mul(ew[:, 0:P], ew[:, 0:P], mask_sb)
                elif grp == 4:
                    # blocks: si4 at 0:4 (t = 512 + 128b), si5 at 4:7
                    # (t = 640 + 128(b-4)); no garbage block.
                    ps = ps_big.tile([P, 7, P], f32, tag="ps",
                                     name=f"sc{i}g4")
                    nc.tensor.matmul(
                        ps[:, 0:4, :],
                        B_sb[i][0:HS, 4 * P:5 * P],
                        A_sb[i][0:HS, TCH:T],
                        start=True,
                        stop=True,
                    )
                    nc.tensor.matmul(
                        ps[:, 4:7, :],
                        B_sb[i][0:HS, 5 * P:6 * P],
                        A_sb[i][0:HS, 5 * P:T],
                        start=True,
                        stop=True,
                    )
                    ew = expwp.tile([P, 7, P], bf16, tag="ew4",
                                    name=f"ew{i}g4")
                    nc.scalar.activation(ew, ps, Exp, scale=0.125)
                    # diagonals: si4 at block 0, si5 at block 4
                    nc.gpsimd.tensor_mul(
                        ew[:, 0:5:4, :],
                        ew[:, 0:5:4, :],
                        mask_sb[:, None, :].to_broadcast([P, 2, P]),
                    )
                else:
                    # blocks: si6 at 0:2 (t = 768 + 128b), si7 at 2 (t=896+)
                    ps = ps_big.tile([P, 3, P], f32, tag="ps",
                                     name=f"sc{i}g5")
                    nc.tensor.matmul(
                        ps[:, 0:2, :],
                        B_sb[i][0:HS, 6 * P:7 * P],
                        A_sb[i][0:HS, 6 * P:T],
                        start=True,
                        stop=True,
                    )
                    nc.tensor.matmul(
                        ps[:, 2, :],
                        B_sb[i][0:HS, 7 * P:T],
                        A_sb[i][0:HS, 7 * P:T],
                        start=True,
                        stop=True,
                    )
                    ew = expwp.tile([P, 3, P], bf16, tag="ew5",
                                    name=f"ew{i}g5")
                    nc.scalar.activation(ew, ps, Exp, scale=0.125)
                    # diagonals: si6 at block 0, si7 at block 2
                    nc.gpsimd.tensor_mul(
                        ew[:, 0:3:2, :],
                        ew[:, 0:3:2, :],
                        mask_sb[:, None, :].to_broadcast([P, 2, P]),
                    )
                expw[(i, grp)] = ew

            def ew_chunk(i, si, ti):
                """128-wide ew column chunk for (si, ti)."""
                if si < 4:
                    ew = expw[(i, si)]
                    c0 = (ti - si) * P
                    return ew[:, c0:c0 + P]
                if si == 4:
                    return expw[(i, 4)][:, ti - 4, :]
                if si == 5:
                    return expw[(i, 4)][:, ti - 1, :]
                if si == 6:
                    return expw[(i, 5)][:, ti - 6, :]
                return expw[(i, 5)][:, 2, :]

            def emit_pv(i, tis):
                for ti in tis:
                    half = ti // 4
                    tii = ti % 4
                    if (i, half) not in po:
                        po[(i, half)] = ps_sm.tile(
                            [P, 4, HS + 1], f32, tag="sm", name=f"po{i}h{half}"
                        )
                    p = po[(i, half)]
                    for si in range(ti + 1):
                        nc.tensor.matmul(
                            p[:, tii, :],
                            ew_chunk(i, si, ti),
                            vaug[i][:, si, :],
                            start=(si == 0),
                            stop=(si == ti),
                        )

            def emit_norm_out(i, half):
                p = po[(i, half)]
                o = outp.tile([P, 4, HS], f32, tag="osb", name=f"o{i}h{half}")
                r = outp.tile([P, 4], f32, tag="recip", name=f"r{i}h{half}")
                nc.vector.reciprocal(r, p[:, :, HS])
                nc.vector.tensor_tensor(
                    o, p[:, :, 0:HS],
                    r[:, :, None].to_broadcast([P, 4, HS]),
                    mybir.AluOpType.mult,
                )
                osb[(i, half)] = o
                dst = out[i].rearrange("(ti p) h -> p ti h", p=P)
                nc.sync.dma_start(dst[:, half * 4:(half + 1) * 4, :], o)

            # ---- software-pipelined emission ----------------------------
            # PE order is chosen so the scalar engine (exp; the pacing
            # engine) never waits: score groups are back-to-back, the next
            # item's first two score groups are emitted right after this
            # item's last, and PV / V-projection fill the PE slack.
            emit_projA(0)
            emit_projB(0)
            emit_scores(0, 0)
            emit_scores(0, 1)
            emit_scores(0, 2)
            emit_scores(0, 3)
            emit_projA(1)
            emit_scores(0, 4)
            emit_projB(1)
            emit_scores(0, 5)
            for i in range(BPC):
                # entering here: scores(i, 0..5) all emitted; A/B(i+1)
                # emitted; PV(i), V(i), norms(i), scores(i+1, *),
                # A/B(i+2) still to do.
                nxt = i + 1 < BPC
                if nxt:
                    emit_scores(i + 1, 0)
                    emit_scores(i + 1, 1)
                if i + 2 < BPC:
                    emit_projA(i + 2)
                emit_projV(i)
                emit_pv(i, [0, 1, 2, 3])
                emit_norm_out(i, 0)
                emit_pv(i, [4, 5])
                if nxt:
                    emit_scores(i + 1, 2)
                    emit_scores(i + 1, 3)
                if i + 2 < BPC:
                    emit_projB(i + 2)
                emit_pv(i, [6, 7])
                emit_norm_out(i, 1)
                if nxt:
                    emit_scores(i + 1, 4)
                    emit_scores(i + 1, 5)

    nc.compile()
    return nc


def _get_nc():
    nc = _cached.get("nc")
    if nc is None:
        nc = _build()
        _cached["nc"] = nc
    return nc


def _in_maps(x, Wk, Wq, Wv):
    bf = ml_dtypes.bfloat16
    x = np.asarray(x, dtype=np.float32)
    Wk = np.asarray(Wk, dtype=np.float32)
    Wq = np.asarray(Wq, dtype=np.float32)
    Wv = np.asarray(Wv, dtype=np.float32)
    # packed per-partition weight blob [P, 768]:
    # [wA(co0)|wA(co1)|wB(co0)|wB(co1)|wV(co0)|wV(co1)|mask]
    # where wX(co) is W{X}.T[co*128:(co+1)*128, :] laid out so partition p
    # holds contraction row p of chunk co.
    wA = np.concatenate([Wq.T, Wk.T], axis=1)   # [C, 2HS]
    wB = np.concatenate([Wk.T, Wq.T], axis=1)   # [C, 2HS]
    wV = Wv.T                                   # [C, HS]
    m = np.triu(np.ones((P, P), dtype=np.float32))
    wcat = np.concatenate(
        [
            wA[0:P, :], wA[P:C, :],
            wB[0:P, :], wB[P:C, :],
            wV[0:P, :], wV[P:C, :],
            m,
        ],
        axis=1,
    )
    wcat = np.ascontiguousarray(wcat).astype(bf)
    maps = []
    for c in range(NCORES):
        xs = x[c * BPC:(c + 1) * BPC]
        xsT = np.ascontiguousarray(xs.transpose(0, 2, 1)).astype(bf)
        maps.append({"xT": xsT, "wcat": wcat})
    return maps


def _run(x, Wk, Wq, Wv, **spmd_kwargs):
    from concourse.bass_utils import run_bass_kernel_spmd

    nc = _get_nc()
    res = run_bass_kernel_spmd(
        nc, _in_maps(x, Wk, Wq, Wv), core_ids=list(range(NCORES)), **spmd_kwargs
    )
    full = np.concatenate([r["out"] for r in res.results], axis=0)
    return full, res


def kernel(x, Wk, Wq, Wv):
    full, _ = _run(x, Wk, Wq, Wv)
    return full
